# revision 5
# baseline (speedup 1.0000x reference)
"""3-layer GraphConv + global mean pool on 8 Trainium2 NeuronCores.

Self-contained: hardcodes N=50000, E=800000, D=H=128, G=64, 8 cores.

Sharding: nodes split into 8 contiguous chunks of 6250 (padded to 6272 =
49 tiles of 128). Each core owns its nodes' rows of h. Per layer:
  z = h @ W_rel (local, fp16)  ->  AllGather z (halo exchange)
  agg^T accumulated per 128-node tile in PSUM via selection-matrix
  matmuls over dma_gather'ed z[src] rows (edges sorted by dst, padded
  to uniform per-(tile,half) chunk counts across cores)
  h' = relu(agg^T + W_root^T @ h^T + b)
Pooling: PE-transpose h3, one-hot graph matmul with appended ones col
(sums+counts in one matmul), AllReduce, divide on-chip.
"""

import numpy as np

import concourse.bacc as bacc
import concourse.mybir as mybir
import concourse.tile as tile
from concourse import library_config
from concourse.bass_utils import run_bass_kernel_spmd
from concourse.masks import make_identity
from concourse.tile import add_dep_helper

N = 50000
E = 800000
D = 128
G = 64
C = 8
NPC = N // C              # 6250 nodes per core
NTILES = 49               # ceil(6250/128)
NPAD = NTILES * 128       # 6272
RPAD = C * NPAD           # 50176 padded global rows
TSPLIT = 25088            # lo/hi gather base split (int16 idx limit)
GROUPS = [list(range(g * 7, (g + 1) * 7)) for g in range(7)]  # 7 groups x 7 tiles

F32 = mybir.dt.float32
F16 = mybir.dt.float16
I16 = mybir.dt.int16


def _preprocess(edge_index, batch):
    """Build per-core gather-index / selection data (pure index math)."""
    src = np.asarray(edge_index[0], dtype=np.int64)
    dst = np.asarray(edge_index[1], dtype=np.int64)
    batch = np.asarray(batch, dtype=np.int64)

    owner = dst // NPC
    dl = dst % NPC                      # dst local node id
    tl = dl // 128                      # dst tile
    pos = dl % 128                      # dst position within tile
    sp = (src // NPC) * NPAD + (src % NPC)   # src row in padded layout
    half = (sp >= TSPLIT).astype(np.int64)

    # counts per (core, tile, half) -> uniform chunk counts K[t,h]
    key = (owner * NTILES + tl) * 2 + half
    cnt = np.bincount(key, minlength=C * NTILES * 2).reshape(C, NTILES, 2)
    K = -(-cnt.max(axis=0) // 128) * 128          # [NTILES, 2] slots, 128-mult
    K = np.maximum(K, 128)

    # slot layout:
    #  idx blocks per (group, half): tiles of the group concatenated
    #  dstv chunk columns in consumption order (g, t, h, c)
    idx_base = np.zeros((NTILES, 2), dtype=np.int64)   # global slot offset
    blk_off = {}                                       # (g,h) -> slot offset
    blk_size = {}
    off = 0
    for g, tiles in enumerate(GROUPS):
        for h in (0, 1):
            blk_off[(g, h)] = off
            for t in tiles:
                idx_base[t, h] = off
                off += K[t, h]
            blk_size[(g, h)] = off - blk_off[(g, h)]
    slot_total = off

    dstv_base = np.zeros((NTILES, 2), dtype=np.int64)  # chunk col offset
    qoff = 0
    for g, tiles in enumerate(GROUPS):
        for t in tiles:
            for h in (0, 1):
                dstv_base[t, h] = qoff
                qoff += K[t, h] // 128
    tot_chunks = qoff
    tot_cols = slot_total // 16

    # order edges by (owner, tile, half), stable
    order = np.argsort(key, kind="stable")
    src_o, sp_o, pos_o, key_o = src[order], sp[order], pos[order], key[order]

    idx_arrs, dstv_arrs, batch_arrs = [], [], []
    # per-(core,tile,half) segment boundaries in the ordered arrays
    seg_start = np.zeros(C * NTILES * 2 + 1, dtype=np.int64)
    np.cumsum(np.bincount(key_o, minlength=C * NTILES * 2), out=seg_start[1:])

    for c in range(C):
        idx_slots = np.zeros(slot_total, dtype=np.int64)
        dst_slots = -np.ones(slot_total, dtype=np.float32)
        for t in range(NTILES):
            for h in (0, 1):
                k = (c * NTILES + t) * 2 + h
                a, b = seg_start[k], seg_start[k + 1]
                n = b - a
                base = idx_base[t, h]
                v = sp_o[a:b] - (TSPLIT if h else 0)
                idx_slots[base:base + n] = v
                dst_slots[base:base + n] = pos_o[a:b]
        assert idx_slots.max() < 32768 and idx_slots.min() >= 0

        # pack idx: [16, cols] wrapped, replicated to 128 partitions
        idx16 = idx_slots.astype(np.int16).reshape(slot_total // 16, 16).T  # [16, cols]
        idx_pack = np.tile(idx16, (8, 1))                                   # [128, cols]
        idx_arrs.append(np.ascontiguousarray(idx_pack))

        # dstv: column q (consumption order) holds the 128 dst positions;
        # chunk (t,h,c) slots live at idx_base[t,h] + 128*c
        dstv = np.empty((128, tot_chunks), dtype=np.float32)
        for t in range(NTILES):
            for h in (0, 1):
                nch = K[t, h] // 128
                s = idx_base[t, h]
                q = dstv_base[t, h]
                dstv[:, q:q + nch] = dst_slots[s:s + K[t, h]].reshape(nch, 128).T
        dstv_arrs.append(dstv)

        bl = batch[c * NPC:(c + 1) * NPC].astype(np.float32)
        bl = np.concatenate([bl, -np.ones(NPAD - NPC, np.float32)])
        batch_arrs.append(np.ascontiguousarray(bl.reshape(NTILES, 128).T))  # [128,49]

    meta = dict(K=K, idx_base=idx_base, dstv_base=dstv_base,
                blk_off=blk_off, blk_size=blk_size,
                slot_total=slot_total, tot_chunks=tot_chunks, tot_cols=tot_cols)
    return idx_arrs, dstv_arrs, batch_arrs, meta


def _build(meta):
    K = meta["K"]
    blk_off, blk_size = meta["blk_off"], meta["blk_size"]
    tot_cols, tot_chunks = meta["tot_cols"], meta["tot_chunks"]
    s_max = max(blk_size.values())

    nc = bacc.Bacc("TRN2", target_bir_lowering=False, debug=False, num_devices=C)

    xT_in = nc.dram_tensor("xT_in", [128, NPAD], F32, kind="ExternalInput")
    idx_in = nc.dram_tensor("idx_in", [128, tot_cols], I16, kind="ExternalInput")
    dstv_in = nc.dram_tensor("dstv_in", [128, tot_chunks], F32, kind="ExternalInput")
    batch_in = nc.dram_tensor("batch_in", [128, NTILES], F32, kind="ExternalInput")
    iota_in = nc.dram_tensor("iota_in", [128, 128], F32, kind="ExternalInput")
    iota64_in = nc.dram_tensor("iota64_in", [128, 64], F32, kind="ExternalInput")
    w_ins = {}
    for l in (1, 2, 3):
        w_ins[f"wrel{l}"] = nc.dram_tensor(f"wrel{l}", [128, 128], F32, kind="ExternalInput")
        w_ins[f"wroot{l}"] = nc.dram_tensor(f"wroot{l}", [128, 128], F32, kind="ExternalInput")
        w_ins[f"b{l}"] = nc.dram_tensor(f"b{l}", [128, 1], F32, kind="ExternalInput")
    pooled_out = nc.dram_tensor("pooled_out", [64, 128], F32, kind="ExternalOutput")
    hand_out = nc.dram_tensor("hand_out", [1, 128], F32, kind="ExternalOutput")

    with tile.TileContext(nc) as tc:
        with (
            tc.tile_pool(name="persist", bufs=1) as pp,
            tc.tile_pool(name="work", bufs=3) as wp,
            tc.tile_pool(name="mtp", bufs=6) as mtp,
            tc.tile_pool(name="yp", bufs=2) as yp,
            tc.tile_pool(name="psum", bufs=2, space="PSUM") as psp,
            tc.tile_pool(name="accps", bufs=1, space="PSUM") as accps,
            tc.tile_pool(name="dram", bufs=1, space="DRAM") as dp,
        ):
            nc.gpsimd.load_library(library_config.mlp)

            # ---- persistent SBUF ----
            hbuf = [pp.tile([128, NPAD], F32, tag=f"hbuf{i}", name=f"hbuf{i}") for i in range(2)]
            h16 = pp.tile([128, NPAD], F16, tag="h16")
            idx_sb = pp.tile([128, tot_cols], I16, tag="idx")
            dstv_sb = pp.tile([128, tot_chunks], F32, tag="dstv")
            batch_sb = pp.tile([128, NTILES], F32, tag="batch")
            iota_sb = pp.tile([128, 128], F32, tag="iota")
            iota64_sb = pp.tile([128, 64], F32, tag="iota64")
            ident_sb = pp.tile([128, 128], F32, tag="ident")
            ones16_sb = pp.tile([128, 1], F16, tag="ones16")
            colsum_sb = pp.tile([128, 1], F32, tag="colsum")
            w16 = {}
            b_sb = {}
            for l in (1, 2, 3):
                for kind in ("wrel", "wroot"):
                    w16[f"{kind}{l}"] = pp.tile([128, 128], F16, tag=f"{kind}{l}", name=f"w16_{kind}{l}")
                b_sb[l] = pp.tile([128, 1], F32, tag=f"b{l}", name=f"bsb{l}")

            nc.sync.dma_start(hbuf[0][:], xT_in[:])
            idx_dma = nc.sync.dma_start(idx_sb[:], idx_in[:])
            nc.sync.dma_start(dstv_sb[:], dstv_in[:])
            nc.sync.dma_start(batch_sb[:], batch_in[:])
            nc.sync.dma_start(iota_sb[:], iota_in[:])
            nc.sync.dma_start(iota64_sb[:], iota64_in[:])
            make_identity(nc, ident_sb[:])
            nc.vector.memset(ones16_sb[:], 1.0)
            for l in (1, 2, 3):
                for kind in ("wrel", "wroot"):
                    wtmp = wp.tile([128, 128], F32, tag="wtmp")
                    nc.sync.dma_start(wtmp[:], w_ins[f"{kind}{l}"][:])
                    nc.vector.tensor_copy(w16[f"{kind}{l}"][:], wtmp[:])
                nc.sync.dma_start(b_sb[l][:], w_ins[f"b{l}"][:])

            # handcrafted partial: column sums of local x (pads are 0)
            nc.vector.tensor_reduce(colsum_sb[:], hbuf[0][:],
                                    mybir.AxisListType.X, mybir.AluOpType.add)

            # ---- DRAM buffers ----
            z_in = [dp.tile([NPAD, 128], F16, tag=f"zin{i}", name=f"zin{i}") for i in range(3)]
            z_full = [dp.tile([RPAD, 128], F16, tag=f"zfull{i}", name=f"zfull{i}",
                               addr_space="Shared") for i in range(3)]
            red_in = dp.tile([65, 129], F32, tag="redin")
            red_out = dp.tile([65, 129], F32, tag="redout", addr_space="Shared")

            gathers_by_layer = {0: [], 1: [], 2: []}

            for li, l in enumerate((1, 2, 3)):
                hprev = hbuf[li % 2]
                hnext = hbuf[(li + 1) % 2]
                zi, zf = z_in[li], z_full[li]

                nc.vector.tensor_copy(h16[:], hprev[:])  # fp32 -> fp16

                # z = h @ W_rel, row-major fp16, per tile
                z_dmas = []
                for t in range(NTILES):
                    z_ps = psp.tile([128, 128], F32, tag="zps", space="PSUM")
                    nc.tensor.matmul(z_ps[:], lhsT=h16[:, t * 128:(t + 1) * 128],
                                     rhs=w16[f"wrel{l}"][:], start=True, stop=True)
                    z_sb = wp.tile([128, 128], F16, tag="zsb")
                    nc.vector.tensor_copy(z_sb[:], z_ps[:])
                    z_dmas.append(nc.sync.dma_start(zi[t * 128:(t + 1) * 128, :], z_sb[:]))

                cc = nc.gpsimd.collective_compute(
                    "AllGather", mybir.AluOpType.bypass,
                    replica_groups=[list(range(C))],
                    ins=[zi[:].opt()], outs=[zf[:].opt()])
                for d in z_dmas:
                    add_dep_helper(cc.ins, d.ins, sync=True, reason="cc after z dma")

                for g, tiles in enumerate(GROUPS):
                    ybufs = {}
                    for h in (0, 1):
                        s_gh = blk_size[(g, h)]
                        o_gh = blk_off[(g, h)]
                        yt = yp.tile([128, s_max // 128, 128], F16, tag=f"y{h}")
                        src_ap = zf[TSPLIT:, :] if h else zf[:, :]
                        gi = nc.gpsimd.dma_gather(
                            yt[:, :s_gh // 128, :], src_ap,
                            idx_sb[:, o_gh // 16:(o_gh + s_gh) // 16],
                            s_gh, s_gh, 128, single_packet=False)
                        add_dep_helper(gi.ins, cc.ins, sync=True, reason="gather after cc")
                        add_dep_helper(gi.ins, idx_dma.ins, sync=True, reason="gather after idx")
                        gathers_by_layer[li].append(gi)
                        ybufs[h] = yt

                    for t in tiles:
                        pre_ps = psp.tile([128, 128], F32, tag="pre", space="PSUM")
                        first = True
                        for h in (0, 1):
                            loc0 = (meta["idx_base"][t, h] - blk_off[(g, h)]) // 128
                            q0 = meta["dstv_base"][t, h]
                            for cch in range(K[t, h] // 128):
                                mt = mtp.tile([128, 128], F16, tag="mt")
                                nc.vector.tensor_scalar(
                                    out=mt[:], in0=iota_sb[:],
                                    scalar1=dstv_sb[:, q0 + cch:q0 + cch + 1],
                                    scalar2=None, op0=mybir.AluOpType.is_equal)
                                nc.tensor.matmul(
                                    pre_ps[:], lhsT=ybufs[h][:, loc0 + cch, :],
                                    rhs=mt[:], start=first, stop=False)
                                first = False
                        nc.tensor.matmul(pre_ps[:], lhsT=w16[f"wroot{l}"][:],
                                         rhs=h16[:, t * 128:(t + 1) * 128],
                                         start=first, stop=True)
                        nc.scalar.activation(hnext[:, t * 128:(t + 1) * 128], pre_ps[:],
                                             mybir.ActivationFunctionType.Relu,
                                             bias=b_sb[l][:, :1])

            # ---- pooling on h3 (in hbuf[1]) ----
            h3T = hbuf[1]
            pool_ps = accps.tile([64, 129], F32, tag="poolps", space="PSUM")
            for t in range(NTILES):
                tr_ps = psp.tile([128, 128], F32, tag="tr", space="PSUM")
                nc.tensor.transpose(tr_ps[:], h3T[:, t * 128:(t + 1) * 128], ident_sb[:])
                haug = wp.tile([128, 129], F16, tag="haug")
                nc.vector.tensor_copy(haug[:, :128], tr_ps[:])
                nc.vector.tensor_copy(haug[:, 128:129], ones16_sb[:])
                mb = wp.tile([128, 64], F16, tag="mb")
                nc.vector.tensor_scalar(
                    out=mb[:], in0=iota64_sb[:], scalar1=batch_sb[:, t:t + 1],
                    scalar2=None, op0=mybir.AluOpType.is_equal)
                nc.tensor.matmul(pool_ps[:], lhsT=mb[:], rhs=haug[:],
                                 start=(t == 0), stop=(t == NTILES - 1))

            red_sb = wp.tile([64, 129], F32, tag="redsb")
            nc.vector.tensor_copy(red_sb[:], pool_ps[:])
            nc.sync.dma_start(red_in[0:64, :], red_sb[:])
            # colsum [128,1] -> [1,128] via PE transpose
            cs_ps = psp.tile([1, 128], F32, tag="tr", space="PSUM")
            nc.tensor.transpose(cs_ps[:], colsum_sb[:, :1], ident_sb[:])
            cs_row = wp.tile([1, 128], F32, tag="csrow")
            nc.vector.tensor_copy(cs_row[:], cs_ps[:])
            nc.sync.dma_start(red_in[64:65, 0:128], cs_row[:])

            ar = nc.gpsimd.collective_compute(
                "AllReduce", mybir.AluOpType.add,
                replica_groups=[list(range(C))],
                ins=[red_in[:].opt()], outs=[red_out[:].opt()])

            fin = wp.tile([128, 129], F32, tag="fin")
            nc.sync.dma_start(fin[:65, :], red_out[:])
            nc.vector.tensor_scalar_max(fin[0:64, 128:129], fin[0:64, 128:129], 1.0)
            rec = wp.tile([128, 1], F32, tag="rec")
            nc.vector.reciprocal(rec[0:64, :1], fin[0:64, 128:129])
            pooled_sb = wp.tile([64, 128], F32, tag="pooledsb")
            nc.vector.tensor_scalar_mul(pooled_sb[:], fin[0:64, 0:128], rec[0:64, :1])
            nc.sync.dma_start(pooled_out[:], pooled_sb[:])

            tot = wp.tile([128, 1], F32, tag="tot")
            nc.vector.tensor_reduce(tot[64:65, :1], fin[64:65, 0:128],
                                    mybir.AxisListType.X, mybir.AluOpType.add)
            nc.vector.reciprocal(tot[64:65, :1], tot[64:65, :1])
            hand_sb = wp.tile([128, 128], F32, tag="handsb")
            nc.vector.tensor_scalar_mul(hand_sb[64:65, :], fin[64:65, 0:128],
                                        tot[64:65, :1])
            nc.sync.dma_start(hand_out[0:1, :], hand_sb[64:65, :])

    nc.compile()
    return nc


def kernel(x, edge_index, batch,
           W_rel1, W_root1, b1, W_rel2, W_root2, b2, W_rel3, W_root3, b3):
    x = np.asarray(x, dtype=np.float32)
    idx_arrs, dstv_arrs, batch_arrs, meta = _preprocess(edge_index, batch)
    nc = _build(meta)

    iota = np.tile(np.arange(128, dtype=np.float32)[None, :], (128, 1))
    iota64 = np.tile(np.arange(64, dtype=np.float32)[None, :], (128, 1))
    common = {
        "iota_in": iota, "iota64_in": iota64,
        "wrel1": np.asarray(W_rel1, np.float32), "wroot1": np.asarray(W_root1, np.float32),
        "wrel2": np.asarray(W_rel2, np.float32), "wroot2": np.asarray(W_root2, np.float32),
        "wrel3": np.asarray(W_rel3, np.float32), "wroot3": np.asarray(W_root3, np.float32),
        "b1": np.asarray(b1, np.float32).reshape(128, 1),
        "b2": np.asarray(b2, np.float32).reshape(128, 1),
        "b3": np.asarray(b3, np.float32).reshape(128, 1),
    }
    in_maps = []
    for c in range(C):
        xT = np.zeros((128, NPAD), dtype=np.float32)
        xT[:, :NPC] = x[c * NPC:(c + 1) * NPC].T
        in_maps.append({
            **common,
            "xT_in": xT,
            "idx_in": idx_arrs[c],
            "dstv_in": dstv_arrs[c],
            "batch_in": batch_arrs[c],
        })

    res = run_bass_kernel_spmd(nc, in_maps, list(range(C)))
    pooled = res.results[0]["pooled_out"].astype(np.float32)
    hand = res.results[0]["hand_out"].reshape(128).astype(np.float32)
    return pooled, hand
